# revision 23
# baseline (speedup 1.0000x reference)
"""Multi-head attention (QKV proj + RoPE + softmax attention) on 8 Trainium2
NeuronCores, tensor-parallel over heads (2 heads per core).

v4: fused single-pass schedule. The exp() of softmax costs ~126us on the
ACT engine per core while projections cost ~164us of pure PE time, so the
two-phase baseline was ACT-bound in phase 2 with ACT idle in phase 1.
This version processes batch-0 chunks first, then interleaves batch-0
attention groups (scores/exp/PV/denominator) between batch-1 projection
units so the exp stream hides under projection matmuls; batch-1 attention
drains at the end. PSUM: 2 proj banks (ping-pong, q/k/v sequential per
head) + 4 score banks + PV + denominator = exactly 8.

Contract: kernel(**inputs) takes the FULL unsharded inputs and returns the
FULL [B, S, H] float32 output.
"""

from contextlib import ExitStack

import numpy as np

B, S, H = 2, 2048, 2048
NH, D = 16, 128
ROPE_BASE = 10000.0
NCORES = 8
HPC = NH // NCORES          # heads per core
CH = HPC * D                # output channels per core
BS = B * S                  # flattened tokens
KT = H // 128               # contraction k-tiles
NCH = BS // 512             # 512-wide token chunks
SKT = S // 128              # score k-tiles per sequence
SQC = S // 512              # query chunks per sequence
CPB = NCH // B              # chunks per batch

LAST_RESULT = None          # BassKernelResults of the most recent run (for test.py)


def _build_nc(with_bias):
    import concourse.mybir as mybir
    import concourse.tile as tile
    from concourse import bacc

    F32 = mybir.dt.float32
    F32R = mybir.dt.float32r
    BF16 = mybir.dt.bfloat16
    AF = mybir.ActivationFunctionType
    ALU = mybir.AluOpType
    ISCALE = float(1.0 / np.sqrt(D))

    nc = bacc.Bacc("TRN2", debug=False, enable_partition_id=False)

    hsT_d = nc.dram_tensor("hsT", [H, BS], BF16, kind="ExternalInput").ap()
    wT_d = {
        p: nc.dram_tensor(f"w{p}T", [H, CH], BF16, kind="ExternalInput").ap()
        for p in "qkv"
    }
    b_d = {
        p: nc.dram_tensor(f"b{p}", [1, CH], F32R, kind="ExternalInput").ap()
        for p in "qkv"
    }
    cos_d = nc.dram_tensor("cosT", [D // 2, S], F32, kind="ExternalInput").ap()
    sin_d = nc.dram_tensor("sinT", [D // 2, S], F32, kind="ExternalInput").ap()
    out_d = nc.dram_tensor("out", [BS, CH], F32, kind="ExternalOutput").ap()

    with tile.TileContext(nc) as tc, ExitStack() as ctx:
        # ---- persistent state ----
        persist = ctx.enter_context(tc.tile_pool(name="persist", bufs=1))
        qT = [persist.tile([128, BS], BF16, tag=f"qT{m}", name=f"qT{m}") for m in range(HPC)]
        kTt = [persist.tile([128, BS], BF16, tag=f"kT{m}", name=f"kT{m}") for m in range(HPC)]
        vN = [persist.tile([128, BS // 128, D], BF16, tag=f"v{m}", name=f"vn{m}") for m in range(HPC)]

        consts = ctx.enter_context(tc.tile_pool(name="consts", bufs=1))
        ones_c = consts.tile([128, 32], BF16, tag="ones_c")
        nc.vector.memset(ones_c, 1.0)
        if with_bias:
            ones_row = consts.tile([1, 512], F32, tag="ones_row")
            nc.vector.memset(ones_row, 1.0)
            b_sb = {}
            for p in "qkv":
                b_sb[p] = consts.tile([1, CH], F32R, tag=f"b{p}", name=f"b{p}sb")
                nc.sync.dma_start(b_sb[p], b_d[p])

        wpool = ctx.enter_context(tc.tile_pool(name="wpool", bufs=1))
        tabs = ctx.enter_context(tc.tile_pool(name="tabs", bufs=1))
        hstp = ctx.enter_context(tc.tile_pool(name="hstp", bufs=3))
        ropet = ctx.enter_context(tc.tile_pool(name="ropet", bufs=3))
        epool = ctx.enter_context(tc.tile_pool(name="epool", bufs=3))
        opool = ctx.enter_context(tc.tile_pool(name="opool", bufs=2))
        dpool = ctx.enter_context(tc.tile_pool(name="dpool", bufs=2))
        fpool = ctx.enter_context(tc.tile_pool(name="fpool", bufs=1))
        stps = ctx.enter_context(tc.tile_pool(name="stps", bufs=2, space="PSUM"))
        p1ps = ctx.enter_context(tc.tile_pool(name="p1ps", bufs=2, space="PSUM"))
        otps = ctx.enter_context(tc.tile_pool(name="otps", bufs=1, space="PSUM"))
        dnps = ctx.enter_context(tc.tile_pool(name="dnps", bufs=1, space="PSUM"))

        # w/hs live as half-tiles: tile deps are whole-tile granular, so
        # halves let the first projection matmuls start ~2x earlier.
        w_sb = {}
        for p in "qkv":
            for h in range(2):
                w_sb[p, h] = wpool.tile([128, KT // 2, CH], BF16,
                                        tag=f"w{p}{h}", name=f"w{p}{h}sb")
        w_r = {p: wT_d[p].rearrange("(k p) c -> p k c", p=128) for p in "qkv"}
        hsT_r = hsT_d.rearrange("(k p) t -> p k t", p=128)
        cos_sb = tabs.tile([D, S], F32, tag="cos")
        sin_sb = tabs.tile([D, S], F32, tag="sin")

        # ---------------- front DMA ----------------
        hs_tiles = {}

        def load_hs(n):
            tok = slice(n * 512, (n + 1) * 512)
            for h in range(2):
                t = hstp.tile([128, KT // 2, 512], BF16, tag=f"hs{h}",
                              name=f"hs{n}{h}")
                hs_tiles[n, h] = t
                nc.sync.dma_start(t, hsT_r[:, h * 8:(h + 1) * 8, tok])

        def load_w(p):
            for h in range(2):
                nc.sync.dma_start(w_sb[p, h], w_r[p][:, h * 8:(h + 1) * 8, :])

        # issue order = hw-queue FIFO order, so strictly by first-need time:
        # wq/hs0 halves interleaved, wk (needed at +7us of PE time thanks to
        # the q,q,k,k,v,v unit order), hs1, wv, rope tables last (the DVE
        # rope lags the PE by design and tolerates late tables).
        for h in range(2):
            t = hstp.tile([128, KT // 2, 512], BF16, tag=f"hs{h}", name=f"hs0{h}")
            hs_tiles[0, h] = t
            for lo, hi in ((0, 1), (1, 2), (2, 4), (4, 8)):
                nc.sync.dma_start(w_sb["q", h][:, lo:hi, :],
                                  w_r["q"][:, 8 * h + lo:8 * h + hi, :])
                nc.sync.dma_start(t[:, lo:hi, :],
                                  hsT_r[:, 8 * h + lo:8 * h + hi, 0:512])
        load_w("k")
        load_hs(1)
        load_w("v")
        nc.scalar.dma_start(cos_sb[0:64, :], cos_d)
        nc.scalar.dma_start(sin_sb[0:64, :], sin_d)
        nc.vector.tensor_copy(cos_sb[64:128, :], cos_sb[0:64, :])
        nc.vector.tensor_scalar_mul(sin_sb[64:128, :], sin_sb[0:64, :], -1.0)

        # ---------------- projection unit ----------------
        # prj PSUM is a 2-bank ping-pong, so the bank drain must be cheap
        # and dependency-free: always a single psum->sbuf bf16 copy. RoPE
        # then reads the copy off the critical path.
        def proj_unit(n, m, p):
            tok = slice(n * 512, (n + 1) * 512)
            pos = slice((n % SQC) * 512, (n % SQC + 1) * 512)
            mh = slice(m * 128, (m + 1) * 128)
            prj = p1ps.tile([128, 512], F32, tag="prj", name=f"prj{n}{m}{p}")
            for k in range(KT):
                nc.tensor.matmul(
                    prj, w_sb[p, k // 8][:, k % 8, mh],
                    hs_tiles[n, k // 8][:, k % 8, :],
                    start=(k == 0),
                    stop=(k == KT - 1) and not with_bias,
                )
            if with_bias:
                nc.tensor.matmul(
                    prj, b_sb[p][:, mh], ones_row.bitcast(F32R),
                    start=False, stop=True,
                )
            if p == "v":
                # psum -> sbuf bf16, DMA-XBAR transpose into [S, d] layout
                # on the sync ring (the scalar ring carries exp; a transpose
                # there head-of-line-blocks the ACT stream in fused regions)
                vt = ropet.tile([128, 512], BF16, tag="vt")
                nc.vector.tensor_copy(vt, prj)
                nc.sync.dma_start_transpose(vN[m][:, n * 4:(n + 1) * 4, :], vt)
            else:
                # RoPE straight from PSUM (f32): dst = ps*cos + rot(ps)*sinSw
                dst = qT[m] if p == "q" else kTt[m]
                t1 = ropet.tile([128, 512], BF16, tag="t1")
                nc.vector.tensor_tensor(t1, prj, cos_sb[:, pos], op=ALU.mult)
                t2 = ropet.tile([128, 512], BF16, tag="t2")
                nc.vector.tensor_tensor(
                    t2[0:64], prj[64:128], sin_sb[64:128, pos], op=ALU.mult
                )
                nc.vector.tensor_tensor(
                    t2[64:128], prj[0:64], sin_sb[0:64, pos], op=ALU.mult
                )
                nc.vector.tensor_tensor(dst[:, tok], t1, t2, op=ALU.add)

        # ---------------- attention pieces ----------------
        def mk_group(m, b, c):
            st = {}
            sq = slice(b * S + c * 512, b * S + (c + 1) * 512)

            def pk(j):
                def fn():
                    if j == 0:
                        st["e"] = epool.tile([128, SKT * 512], BF16, tag="e",
                                             name=f"e{m}{b}{c}")
                    st_ps = stps.tile([128, 1024], F32, tag="st")
                    for i in range(2):
                        sk = 2 * j + i
                        kblk = kTt[m][:, b * S + sk * 128: b * S + (sk + 1) * 128]
                        nc.tensor.matmul(
                            st_ps[:, i * 512:(i + 1) * 512],
                            kblk, qT[m][:, sq], start=True, stop=True,
                        )
                    nc.scalar.activation(
                        st["e"][:, j * 1024:(j + 1) * 1024], st_ps,
                        AF.Exp, scale=ISCALE,
                    )
                return (0.45, fn)

            def pvh(h):
                def fn():
                    if h == 0:
                        st["ot"] = otps.tile([128, 512], F32, tag="ot", name=f"ot{m}{b}{c}")
                    for sk in range(8 * h, 8 * h + 8):
                        nc.tensor.matmul(
                            st["ot"], vN[m][:, b * SKT + sk, :],
                            st["e"][:, sk * 512:(sk + 1) * 512],
                            start=(sk == 0), stop=(sk == SKT - 1),
                        )
                return (1.75, fn)

            def dn():
                def fn():
                    # 4 col-tiled ones-matmuls per pack run concurrently on
                    # distinct 32-col PE groups
                    dn_ps = dnps.tile([128, 512], F32, tag="dn")
                    for p4 in range(4):
                        for g4 in range(4):
                            sk = 4 * g4 + p4
                            nc.tensor.matmul(
                                dn_ps[32 * g4:32 * (g4 + 1), :], ones_c,
                                st["e"][:, sk * 512:(sk + 1) * 512],
                                start=(p4 == 0), stop=(p4 == 3),
                                tile_position=(0, 32 * g4),
                            )
                    dn_sb = dpool.tile([128, 512], BF16, tag="dn_sb")
                    nc.vector.tensor_copy(dn_sb, dn_ps)
                    dnT = dpool.tile([128, 4, 128], BF16, tag="dnT")
                    nc.sync.dma_start_transpose(dnT, dn_sb)
                    # transposed denominator partials sit at cols {0,32,64,96}
                    sAB = dpool.tile([128, 4, 2], F32, tag="sAB")
                    nc.vector.tensor_tensor(
                        sAB, dnT[:, :, 0:64:32], dnT[:, :, 64:128:32], op=ALU.add
                    )
                    dsum = dpool.tile([128, 4, 1], F32, tag="dsum")
                    nc.vector.tensor_tensor(
                        dsum, sAB[:, :, 0:1], sAB[:, :, 1:2], op=ALU.add
                    )
                    st["rdt"] = dpool.tile([128, 4, 1], F32, tag="rdt",
                                           name=f"rdt{m}{b}{c}")
                    nc.vector.reciprocal(st["rdt"], dsum)
                return (1.0, fn)

            def post():
                def fn():
                    ot_sb = opool.tile([128, 512], BF16, tag="ot_sb")
                    nc.vector.tensor_copy(ot_sb, st["ot"])
                    otT = opool.tile([128, 4, 128], BF16, tag="otT")
                    nc.sync.dma_start_transpose(otT, ot_sb)
                    o_sb = opool.tile([128, 4, 128], F32, tag="o")
                    for blk in range(4):
                        nc.vector.tensor_scalar_mul(
                            o_sb[:, blk, :], otT[:, blk, :], st["rdt"][:, blk, :]
                        )
                    r0 = b * S + c * 512
                    dst = out_d[r0:r0 + 512, m * 128:(m + 1) * 128].rearrange(
                        "(blk p) c -> p blk c", p=128
                    )
                    nc.sync.dma_start(dst, o_sb)
                return (0.5, fn)

            def post_fast():
                # last group only: otT on the scalar ring (exp stream is
                # finished), so it does not serialize behind dnT on sync
                def fn():
                    ot_sb = opool.tile([128, 512], BF16, tag="ot_sb")
                    nc.vector.tensor_copy(ot_sb, st["ot"])
                    otT = opool.tile([128, 4, 128], BF16, tag="otT")
                    nc.scalar.dma_start_transpose(otT, ot_sb)
                    o_sb = opool.tile([128, 4, 128], F32, tag="o")
                    for blk in range(4):
                        nc.vector.tensor_scalar_mul(
                            o_sb[:, blk, :], otT[:, blk, :], st["rdt"][:, blk, :]
                        )
                    r0 = b * S + c * 512
                    dst = out_d[r0:r0 + 512, m * 128:(m + 1) * 128].rearrange(
                        "(blk p) c -> p blk c", p=128
                    )
                    nc.sync.dma_start(dst, o_sb)
                return (0.5, fn)

            # --- finer pieces used only for the very last group ---
            def pvp(j):
                def fn():
                    if j == 0:
                        st["ot"] = otps.tile([128, 512], F32, tag="ot",
                                             name=f"ot{m}{b}{c}")
                    for sk in (2 * j, 2 * j + 1):
                        nc.tensor.matmul(
                            st["ot"], vN[m][:, b * SKT + sk, :],
                            st["e"][:, sk * 512:(sk + 1) * 512],
                            start=(sk == 0), stop=(sk == SKT - 1),
                        )
                return (0.45, fn)

            def dnh(half):
                # column group g4 sums blocks {2g4, 2g4+1} of its half;
                # two concurrent 4-position packs per half
                def fn():
                    if half == 0:
                        st["dnp"] = dnps.tile([128, 512], F32, tag="dn",
                                              name=f"dn{m}{b}{c}")
                    dn_ps = st["dnp"]
                    for j in range(2):
                        for g4 in range(4):
                            sk = 8 * half + 2 * g4 + j
                            nc.tensor.matmul(
                                dn_ps[32 * g4:32 * (g4 + 1), :], ones_c,
                                st["e"][:, sk * 512:(sk + 1) * 512],
                                start=(half == 0 and j == 0),
                                stop=(half == 1 and j == 1),
                                tile_position=(0, 32 * g4),
                            )
                    if half == 1:
                        dn_sb = dpool.tile([128, 512], BF16, tag="dn_sb")
                        nc.vector.tensor_copy(dn_sb, dn_ps)
                        dnT = dpool.tile([128, 4, 128], BF16, tag="dnT")
                        nc.sync.dma_start_transpose(dnT, dn_sb)
                        sAB = dpool.tile([128, 4, 2], F32, tag="sAB")
                        nc.vector.tensor_tensor(
                            sAB, dnT[:, :, 0:64:32], dnT[:, :, 64:128:32],
                            op=ALU.add
                        )
                        dsum = dpool.tile([128, 4, 1], F32, tag="dsum",
                                          name=f"ds{m}{b}{c}")
                        nc.vector.tensor_tensor(
                            dsum, sAB[:, :, 0:1], sAB[:, :, 1:2], op=ALU.add
                        )
                        st["rdt"] = dpool.tile([128, 4, 1], F32, tag="rdt",
                                               name=f"rdt{m}{b}{c}")
                        nc.vector.reciprocal(st["rdt"], dsum)
                return (0.5, fn)

            def posth(h):
                def fn():
                    ot_sb = fpool.tile([128, 256], BF16, tag=f"ot_sbh{h}")
                    nc.vector.tensor_copy(ot_sb, st["ot"][:, h * 256:(h + 1) * 256])
                    otT = fpool.tile([128, 2, 128], BF16, tag=f"otTh{h}")
                    nc.scalar.dma_start_transpose(otT, ot_sb)
                    o_sb = fpool.tile([128, 2, 128], F32, tag=f"oh{h}")
                    for j in range(2):
                        blk = 2 * h + j
                        nc.vector.tensor_scalar_mul(
                            o_sb[:, j, :], otT[:, j, :], st["rdt"][:, blk, :]
                        )
                    r0 = b * S + c * 512 + h * 256
                    dst = out_d[r0:r0 + 256, m * 128:(m + 1) * 128].rearrange(
                        "(blk p) c -> p blk c", p=128
                    )
                    nc.sync.dma_start(dst, o_sb)
                return (0.3, fn)

            return {"pk": pk, "pvh": pvh, "dn": dn, "post": post,
                    "post_fast": post_fast,
                    "pvp": pvp, "dnh": dnh, "posth": posth}

        # work list: scores packs of group g interleaved with the consume
        # pieces of group g-1 (PV halves, then denominator, then output) so
        # the ACT exp stream and PE stay dense with only 4 score banks.
        groups = [(m, b, c) for b in range(B) for m in range(HPC)
                  for c in range(SQC)]
        W = []
        boundary = [None]
        prev = None
        for gi, g in enumerate(groups):
            G = mk_group(*g)
            if g[1] == 1 and boundary[0] is None:
                boundary[0] = len(W)
            pks = [G["pk"](j) for j in range(SKT // 2)]
            if prev is None:
                W += pks
            else:
                P = prev
                W += [pks[0], pks[1], P["pvh"](0), pks[2], P["pvh"](1),
                      pks[3], P["dn"](), pks[4], P["post"](),
                      pks[5], pks[6], pks[7]]
            prev = G
        # final group: per-pack PV and two-phase denominator interleaved
        # with its last score packs, so after the last exp only ~1us of PE
        # work and a short cast+transpose+scale+DMA chain remain. (Its pk5,
        # pk6, pk7 were appended by the loop above; pull them back out.)
        L = prev
        W += [L["pvh"](0), L["dn"](), L["pvh"](1), L["post_fast"]()]
        boundary = boundary[0]

        wi = [0]

        def att_step(budget):
            spent = 0.0
            while wi[0] < boundary and spent < budget:
                cost, fn = W[wi[0]]
                fn()
                wi[0] += 1
                spent += cost

        # ---------------- schedule ----------------
        # stage A: batch-0 chunks, pure projections
        UNIT_ORDER = [(p, m) for p in "qkv" for m in range(HPC)]
        for n in range(CPB):
            if n + 2 < NCH and (n + 2, 0) not in hs_tiles:
                load_hs(n + 2)
            for p, m in UNIT_ORDER:
                proj_unit(n, m, p)
        # stage B: batch-1 chunks with batch-0 attention interleaved.
        # The first two insertion points are skipped: the first attention
        # pieces wait on chunk-3 RoPE, which lands early in chunk 4.
        ui = 0
        for n in range(CPB, NCH):
            if n + 2 < NCH and (n + 2, 0) not in hs_tiles:
                load_hs(n + 2)
            for p, m in UNIT_ORDER:
                proj_unit(n, m, p)
                ui += 1
                if ui > 2:
                    att_step(3.2)
        # stage C: drain everything left (tail of batch 0 + all batch 1)
        while wi[0] < len(W):
            W[wi[0]][1]()
            wi[0] += 1

    nc.compile()
    return nc


def _rope_tables():
    inv_freq = 1.0 / (ROPE_BASE ** (np.arange(0, D, 2, dtype=np.float64) / D))
    pos = np.arange(S, dtype=np.float64)
    ang = pos[:, None] * inv_freq[None, :]          # [S, D/2]
    emb = np.concatenate([ang, ang], axis=-1)       # [S, D]
    # only the 64 distinct rows ship; the kernel mirrors cos into rows
    # 64:128 and writes -sin there (t2[0:64] = q[64:128] * (-sin),
    # t2[64:128] = q[0:64] * (+sin))
    cosT = np.ascontiguousarray(np.cos(emb).T[0:64].astype(np.float32))
    sinT = np.ascontiguousarray(np.sin(emb).T[0:64].astype(np.float32))
    return cosT, sinT


def _ensure_axon_hooks():
    """bass_utils imports antenv.axon_hooks unconditionally when BASS_TRACE
    is set; this image's antenv package lacks that submodule. Synthesize it
    (and register the real NTFF hook when available) so tracing works and a
    bare import can't crash the run."""
    import sys
    import types

    try:
        import antenv.axon_hooks  # noqa: F401
        return
    except ImportError:
        pass
    try:
        import antenv
    except ImportError:
        return

    mod = types.ModuleType("antenv.axon_hooks")
    mod._hook = None
    mod.set_axon_ntff_profile_hook = lambda h: setattr(mod, "_hook", h)
    mod.get_axon_ntff_profile_hook = lambda: mod._hook
    sys.modules["antenv.axon_hooks"] = mod
    antenv.axon_hooks = mod
    try:
        from trn_agent_boot.trn_boot import _ntff_profile_via_ctypes

        mod._hook = _ntff_profile_via_ctypes("/opt/axon/libaxon_pjrt.so")
    except Exception:
        pass


def kernel(hidden_states, Wq, bq, Wk, bk, Wv, bv):
    global LAST_RESULT
    import ml_dtypes

    _ensure_axon_hooks()
    from concourse.bass_utils import run_bass_kernel_spmd

    BF = ml_dtypes.bfloat16
    hs = np.asarray(hidden_states, dtype=np.float32).reshape(BS, H)
    Wq = np.asarray(Wq, dtype=np.float32)
    Wk = np.asarray(Wk, dtype=np.float32)
    Wv = np.asarray(Wv, dtype=np.float32)
    bq = np.asarray(bq, dtype=np.float32)
    bk = np.asarray(bk, dtype=np.float32)
    bv = np.asarray(bv, dtype=np.float32)

    with_bias = bool(np.any(bq) or np.any(bk) or np.any(bv))
    nc = _build_nc(with_bias)

    hsT = np.ascontiguousarray(hs.T.astype(BF))     # [H, BS] bf16
    cosT, sinT = _rope_tables()

    in_maps = []
    for c in range(NCORES):
        ch = slice(c * CH, (c + 1) * CH)
        m = {
            "hsT": hsT,
            "wqT": np.ascontiguousarray(Wq[ch, :].T.astype(BF)),
            "wkT": np.ascontiguousarray(Wk[ch, :].T.astype(BF)),
            "wvT": np.ascontiguousarray(Wv[ch, :].T.astype(BF)),
            "cosT": cosT,
            "sinT": sinT,
        }
        if with_bias:
            m["bq"] = np.ascontiguousarray(bq[None, ch])
            m["bk"] = np.ascontiguousarray(bk[None, ch])
            m["bv"] = np.ascontiguousarray(bv[None, ch])
        else:
            z = np.zeros((1, CH), dtype=np.float32)
            m["bq"] = m["bk"] = m["bv"] = z
        in_maps.append(m)

    res = run_bass_kernel_spmd(nc, in_maps, core_ids=list(range(NCORES)))
    LAST_RESULT = res

    full = np.concatenate([r["out"] for r in res.results], axis=1)  # [BS, H]
    return full.reshape(B, S, H)
